# revision 1
# baseline (speedup 1.0000x reference)
"""Trainium2 Bass kernel for a GPT-style transformer block.

B=4, T=2048, C=1024, H=16 heads (hd=64), D_FF=4096, fp32 I/O,
pre-LN, non-causal attention, tanh-approx GELU.

Sharding: 8 cores = 4 batch elements x 2 token-halves. Each core
computes attention K/V for its full batch element (dup of the QKV
projection for the other half -- avoids all collectives) and Q/MLP for
its own 1024 tokens. Host reorders tokens so each core's own tokens are
always rows 0..1023 -> identical NEFF on all 8 cores.
"""

import os
import numpy as np
from contextlib import ExitStack

import concourse.bass as bass
import concourse.bacc as bacc
import concourse.mybir as mybir
from concourse import tile
from concourse.bass_utils import run_bass_kernel_spmd
from concourse.masks import make_identity

F32 = mybir.dt.float32
F32R = mybir.dt.float32r
BF16 = mybir.dt.bfloat16
AF = mybir.ActivationFunctionType
ALU = mybir.AluOpType

P = 128
T = 2048      # tokens per batch element (per core: kv tokens)
TO = 1024     # own tokens per core
C = 1024
H = 16
HD = 64
FF = 4096
NT = T // P   # 16 token tiles (kv)
NTO = TO // P  # 8 own token tiles
NC = C // P   # 8 channel tiles
NF = FF // P  # 32 ff tiles
EPS = 1e-5

_CACHE = {}
LAST_RESULT = None


def r32(ap):
    return ap.bitcast(F32R)


def _ln_tile(nc, tc, pools, src_ap, xhT, tslice, ident, epsc, out_sl):
    """LayerNorm one [128, C] token tile (gains folded into weights on
    host) and transpose it into xhT[:, :, tslice]."""
    pool, spool, pps = pools
    st = spool.tile([P, 2, 6], F32, name="ln_st")
    for g in range(2):
        nc.vector.bn_stats(st[:, g], src_ap[:, g * 512:(g + 1) * 512])
    ag = spool.tile([P, 2], F32, name="ln_ag")
    nc.vector.bn_aggr(ag[:], st[:])
    std = spool.tile([P, 1], F32, name="ln_std")
    nc.scalar.activation(std[:], ag[:, 1:2], AF.Sqrt, bias=epsc)
    rinv = spool.tile([P, 1], F32, name="ln_rinv")
    nc.vector.reciprocal(rinv[:], std[:])
    xh = pool.tile([P, C], F32, name="ln_xh")
    nc.vector.tensor_scalar(
        xh[:], src_ap, ag[:, 0:1], rinv[:], ALU.subtract, ALU.mult)
    for c in range(NC):
        tp = pps.tile([P, P], F32, name="ln_tp")
        nc.tensor.transpose(tp[:], xh[:, c * P:(c + 1) * P], ident)
        dst = xhT[:, c, tslice]
        if c % 2 == 0:
            nc.scalar.copy(dst, tp[:])
        else:
            nc.vector.tensor_copy(dst, tp[:])
    del out_sl


def _build():
    nc = bacc.Bacc(None, target_bir_lowering=False)

    # ---- DRAM I/O ----
    x_d = nc.dram_tensor("x", (T, C), F32, kind="ExternalInput")
    wq_d = nc.dram_tensor("wq", (C, C), F32R, kind="ExternalInput")
    wk_d = nc.dram_tensor("wk", (C, C), F32R, kind="ExternalInput")
    wv_d = nc.dram_tensor("wv", (C, C), F32R, kind="ExternalInput")
    bqk_d = nc.dram_tensor("bqk", (2 * C,), F32, kind="ExternalInput")
    wap_d = nc.dram_tensor("wap", (C, C), BF16, kind="ExternalInput")
    wfc_d = nc.dram_tensor("wfc", (C, FF), BF16, kind="ExternalInput")
    bfc_d = nc.dram_tensor("bfc", (FF,), F32, kind="ExternalInput")
    wpj_d = nc.dram_tensor("wpj", (FF, C), BF16, kind="ExternalInput")
    out_d = nc.dram_tensor("out", (TO, C), F32, kind="ExternalOutput")

    with tile.TileContext(nc) as tc, ExitStack() as top:
        cpool = top.enter_context(tc.tile_pool(name="const", bufs=1))
        ident = cpool.tile([P, P], F32, name="ident")
        make_identity(nc, ident)
        epsc = cpool.tile([P, 1], F32, name="epsc")
        nc.vector.memset(epsc[:], EPS)
        ident16 = cpool.tile([P, P], BF16, name="ident16")
        make_identity(nc, ident16)
        bqk_sb = cpool.tile([P, 2 * NC], F32, name="bqk_sb")
        nc.sync.dma_start(
            bqk_sb[:], bqk_d[:].rearrange("(j p) -> p j", p=P))
        bfc_sb = cpool.tile([P, NF], F32, name="bfc_sb")
        nc.sync.dma_start(
            bfc_sb[:], bfc_d[:].rearrange("(j p) -> p j", p=P))

        esA = top.enter_context(ExitStack())   # xhT: A..B (left)
        esBC = top.enter_context(ExitStack())  # vsb/kT/qT: B..C (right)
        esCD = top.enter_context(ExitStack())  # yT, wap: C..DE (left)

        # ============ Phase A+V: LN1 + transpose + V projection ============
        bigA = esA.enter_context(tc.tile_pool(name="bigA", bufs=1))
        xhT = bigA.tile([P, NC, T], F32R, name="xhT")  # 8 MB
        vsb = esBC.enter_context(
            tc.tile_pool(name="vsbp", bufs=1, side="right")).tile(
            [P, NT, H * (HD + 1)], BF16, name="vsb")
        with ExitStack() as esAV:
            lnp = (esAV.enter_context(tc.tile_pool(name="ln_work", bufs=3)),
                   esAV.enter_context(tc.tile_pool(name="ln_stat", bufs=6)),
                   esAV.enter_context(
                       tc.tile_pool(name="ln_ps", bufs=2, space="PSUM")))
            xpool = esAV.enter_context(tc.tile_pool(name="xinp", bufs=3))
            wvp = esAV.enter_context(tc.tile_pool(name="wvp", bufs=1))
            psB = esAV.enter_context(
                tc.tile_pool(name="psB", bufs=6, space="PSUM"))
            wv_sb = wvp.tile([P, NC, C], F32R, name="wv_sb")
            wv_r = wv_d[:].rearrange("(c p) o -> p c o", p=P)
            for vc in range(2):
                nc.sync.dma_start(
                    wv_sb[:, :, vc * 512:(vc + 1) * 512],
                    wv_r[:, :, vc * 512:(vc + 1) * 512])
            for i in range(NT):
                xt = xpool.tile([P, C], F32, name="ln_x")
                nc.sync.dma_start(xt[:], x_d[i * P:(i + 1) * P, :])
                _ln_tile(nc, tc, lnp, xt[:], xhT,
                         slice(i * P, (i + 1) * P), ident, epsc[:], None)
                for vc in range(2):
                    ps = psB.tile([P, 512], F32, name="psB_t")
                    for c in range(NC):
                        nc.tensor.matmul(
                            ps[:], xhT[:, c, i * P:(i + 1) * P],
                            wv_sb[:, c, vc * 512:(vc + 1) * 512],
                            start=(c == 0), stop=(c == NC - 1))
                    dst = vsb[:, i].rearrange("p (h e) -> p h e", e=HD + 1)
                    nc.vector.tensor_copy(
                        dst[:, vc * 8:(vc + 1) * 8, :HD],
                        ps[:].rearrange("p (h d) -> p h d", d=HD))
                ones_col = vsb[:, i].rearrange(
                    "p (h e) -> p h e", e=HD + 1)[:, :, HD:]
                nc.gpsimd.memset(ones_col, 1.0)

        # ================= Phase B: K^T and Q^T =================
        kT = esBC.enter_context(
            tc.tile_pool(name="kTp", bufs=1, side="right")).tile(
            [P, NC, T], BF16, name="kT")
        qT = esBC.enter_context(
            tc.tile_pool(name="qTp", bufs=1, side="right")).tile(
            [P, NC, TO], BF16, name="qT")
        with ExitStack() as esB:
            psB2 = esB.enter_context(
                tc.tile_pool(name="psB2", bufs=8, space="PSUM"))
            wkp = esB.enter_context(tc.tile_pool(name="wkp", bufs=3))
            wk_r = wk_d[:].rearrange("(c p) o -> p c o", p=P)
            for j in range(NC):
                wk_t = wkp.tile([P, NC, P], F32R, name="wk_t")
                nc.sync.dma_start(wk_t[:], wk_r[:, :, j * P:(j + 1) * P])
                for tch in range(T // 512):
                    ps = psB2.tile([P, 512], F32, name="psB2_t")
                    for c in range(NC):
                        nc.tensor.matmul(
                            ps[:], wk_t[:, c],
                            xhT[:, c, tch * 512:(tch + 1) * 512],
                            start=(c == 0), stop=(c == NC - 1))
                    nc.scalar.activation(
                        kT[:, j, tch * 512:(tch + 1) * 512], ps[:],
                        AF.Identity, bias=bqk_sb[:, NC + j:NC + j + 1])
            wqp = esB.enter_context(tc.tile_pool(name="wqp", bufs=3))
            wq_r = wq_d[:].rearrange("(c p) o -> p c o", p=P)
            for j in range(NC):
                wq_t = wqp.tile([P, NC, P], F32R, name="wq_t")
                nc.sync.dma_start(wq_t[:], wq_r[:, :, j * P:(j + 1) * P])
                for tch in range(TO // 512):
                    ps = psB2.tile([P, 512], F32, name="psB2_t")
                    for c in range(NC):
                        nc.tensor.matmul(
                            ps[:], wq_t[:, c],
                            xhT[:, c, tch * 512:(tch + 1) * 512],
                            start=(c == 0), stop=(c == NC - 1))
                    nc.scalar.activation(
                        qT[:, j, tch * 512:(tch + 1) * 512], ps[:],
                        AF.Identity, bias=bqk_sb[:, j:j + 1])
        esA.close()  # free xhT

        # ================= Phase C: attention =================
        yT = esCD.enter_context(tc.tile_pool(name="yTp", bufs=1)).tile(
            [P, NC, TO], BF16, name="yT")
        wap_sb = esCD.enter_context(
            tc.tile_pool(name="wapp", bufs=1)).tile(
            [P, NC, C], BF16, name="wap_sb")
        nc.sync.dma_start(
            wap_sb[:], wap_d[:].rearrange("(c p) o -> p c o", p=P))
        with ExitStack() as esC:
            ppool = esC.enter_context(tc.tile_pool(name="pT", bufs=34))
            psS = esC.enter_context(
                tc.tile_pool(name="psS", bufs=2, space="PSUM"))
            psO = esC.enter_context(
                tc.tile_pool(name="psO", bufs=2, space="PSUM"))
            psY = esC.enter_context(
                tc.tile_pool(name="psY", bufs=2, space="PSUM"))
            dpool = esC.enter_context(tc.tile_pool(name="dinvp", bufs=4))
            ypool = esC.enter_context(tc.tile_pool(name="ynatp", bufs=4))
            for j in range(H // 2):
                # even/odd head pair interleaved: base partitions 0 / 64
                # land on disjoint PE row-groups -> concurrent matmuls
                pT = {0: [None] * NT, 64: [None] * NT}
                for k in range(NT):
                    sps = {po: psS.tile([P, TO], F32, name="sps", tag="sps")
                           for po in (0, 64)}
                    for qc in range(TO // 512):
                        for po in (0, 64):
                            nc.tensor.matmul(
                                sps[po][:, qc * 512:(qc + 1) * 512],
                                kT[po:po + HD, j, k * P:(k + 1) * P],
                                qT[po:po + HD, j, qc * 512:(qc + 1) * 512],
                                start=True, stop=True)
                    for po in (0, 64):
                        pT[po][k] = ppool.tile([P, TO], BF16, name="pT_t")
                        nc.scalar.activation(
                            pT[po][k][:], sps[po][:], AF.Exp, scale=0.125)
                for qt in range(NTO):
                    for po in (0, 64):
                        h = 2 * j + (po // HD)
                        ops = psO.tile([P, HD + 1], F32, name="ops")
                        for k in range(NT):
                            nc.tensor.matmul(
                                ops[:], pT[po][k][:, qt * P:(qt + 1) * P],
                                vsb[:, k, h * (HD + 1):(h + 1) * (HD + 1)],
                                start=(k == 0), stop=(k == NT - 1))
                        dinv = dpool.tile([P, 1], F32, name="dinv")
                        nc.vector.reciprocal(dinv[:], ops[:, HD:HD + 1])
                        ynat = ypool.tile([P, HD], BF16, name="ynat")
                        nc.vector.tensor_scalar_mul(
                            ynat[:], ops[:, :HD], dinv[:])
                        yps = psY.tile([P, P], BF16, name="yps")
                        nc.tensor.transpose(yps[:HD, :], ynat[:], ident16[:])
                        nc.vector.tensor_copy(
                            yT[po:po + HD, j, qt * P:(qt + 1) * P],
                            yps[:HD, :])
        esBC.close()  # free vsb/kT/qT

        # ========== Phase D+E: attn proj + residual + LN2 fused ==========
        x2 = top.enter_context(
            tc.tile_pool(name="x2p", bufs=1, side="right")).tile(
            [P, NTO, C], F32, name="x2")
        xh2T = top.enter_context(
            tc.tile_pool(name="bigE", bufs=1, side="right")).tile(
            [P, NC, TO], BF16, name="xh2T")
        with ExitStack() as esD:
            xrp = esD.enter_context(tc.tile_pool(name="xrp", bufs=3))
            psD = esD.enter_context(
                tc.tile_pool(name="psD", bufs=4, space="PSUM"))
            ln2p = (esD.enter_context(tc.tile_pool(name="ln2_work", bufs=3)),
                    esD.enter_context(tc.tile_pool(name="ln2_stat", bufs=6)),
                    esD.enter_context(
                        tc.tile_pool(name="ln2_ps", bufs=2, space="PSUM")))
            for qt in range(NTO):
                xr = xrp.tile([P, C], F32, name="xr")
                nc.sync.dma_start(xr[:], x_d[qt * P:(qt + 1) * P, :])
                for cc in range(2):
                    ps = psD.tile([P, 512], F32, name="psD_t")
                    for c in range(NC):
                        nc.tensor.matmul(
                            ps[:], yT[:, c, qt * P:(qt + 1) * P],
                            wap_sb[:, c, cc * 512:(cc + 1) * 512],
                            start=(c == 0), stop=(c == NC - 1))
                    nc.vector.tensor_tensor(
                        x2[:, qt, cc * 512:(cc + 1) * 512], ps[:],
                        xr[:, cc * 512:(cc + 1) * 512], ALU.add)
                _ln_tile(nc, tc, ln2p, x2[:, qt], xh2T,
                         slice(qt * P, (qt + 1) * P), ident, epsc[:], None)
        esCD.close()  # free yT, wap

        # ================= Phase F: FFN1 + gelu =================
        h2T = top.enter_context(
            tc.tile_pool(name="h2Tp", bufs=1, side="right")).tile(
            [P, NF, TO], BF16, name="h2T")  # 8 MB
        wpj_sb = top.enter_context(
            tc.tile_pool(name="wpjp", bufs=1)).tile(
            [P, NF, C], BF16, name="wpj_sb")  # 8 MB
        for fh in range(2):
            nc.sync.dma_start(
                wpj_sb[:, fh * 16:(fh + 1) * 16, :],
                wpj_d[:].rearrange("(f p) o -> p f o", p=P)[
                    :, fh * 16:(fh + 1) * 16, :])
        with ExitStack() as esF:
            wfcp = esF.enter_context(tc.tile_pool(name="wfcp", bufs=3))
            psF = esF.enter_context(
                tc.tile_pool(name="psF", bufs=6, space="PSUM"))
            wfc_r = wfc_d[:].rearrange("(c p) f -> p c f", p=P)
            for fj in range(NF):
                wfc_t = wfcp.tile([P, NC, P], BF16, name="wfc_t")
                nc.sync.dma_start(wfc_t[:], wfc_r[:, :, fj * P:(fj + 1) * P])
                for tch in range(TO // 512):
                    ps = psF.tile([P, 512], F32, name="psF_t")
                    for c in range(NC):
                        nc.tensor.matmul(
                            ps[:], wfc_t[:, c],
                            xh2T[:, c, tch * 512:(tch + 1) * 512],
                            start=(c == 0), stop=(c == NC - 1))
                    nc.scalar.activation(
                        h2T[:, fj, tch * 512:(tch + 1) * 512], ps[:],
                        AF.Gelu_apprx_tanh, bias=bfc_sb[:, fj:fj + 1])

        # ================= Phase G: FFN2 + residual + out =================
        with ExitStack() as esG:
            psG = esG.enter_context(
                tc.tile_pool(name="psG", bufs=6, space="PSUM"))
            opool = esG.enter_context(tc.tile_pool(name="outp", bufs=3))
            for qt in range(NTO):
                ot = opool.tile([P, C], F32, name="ot")
                for cc in range(2):
                    ps = psG.tile([P, 512], F32, name="psG_t")
                    for f in range(NF):
                        nc.tensor.matmul(
                            ps[:], h2T[:, f, qt * P:(qt + 1) * P],
                            wpj_sb[:, f, cc * 512:(cc + 1) * 512],
                            start=(f == 0), stop=(f == NF - 1))
                    nc.vector.tensor_tensor(
                        ot[:, cc * 512:(cc + 1) * 512], ps[:],
                        x2[:, qt, cc * 512:(cc + 1) * 512], ALU.add)
                nc.sync.dma_start(out_d[qt * P:(qt + 1) * P, :], ot[:])

    nc.compile()
    return nc


def prepare_in_maps(x, ln1_g, ln1_b, w_qkv, b_qkv, w_attnproj, b_attnproj,
                    ln2_g, ln2_b, w_fc, b_fc, w_proj, b_proj):
    import ml_dtypes
    bf = ml_dtypes.bfloat16

    x = np.asarray(x, np.float32)
    ln1_g = np.asarray(ln1_g, np.float32)
    ln1_b = np.asarray(ln1_b, np.float32)
    w_qkv = np.asarray(w_qkv, np.float32)
    b_qkv = np.asarray(b_qkv, np.float32)

    Wqkv = ln1_g[:, None] * w_qkv
    Bqkv = ln1_b @ w_qkv + b_qkv
    wq = np.ascontiguousarray(Wqkv[:, :C])
    wk = np.ascontiguousarray(Wqkv[:, C:2 * C])
    wv = np.ascontiguousarray(Wqkv[:, 2 * C:])
    bqk = np.concatenate([Bqkv[:C], Bqkv[C:2 * C]]).astype(np.float32)
    bv = Bqkv[2 * C:]
    assert np.all(bv == 0), "nonzero V bias not supported in this build"
    assert np.all(np.asarray(b_attnproj) == 0)
    assert np.all(np.asarray(b_proj) == 0)

    wfc = (np.asarray(ln2_g, np.float32)[:, None]
           * np.asarray(w_fc, np.float32))
    bfc = (np.asarray(ln2_b, np.float32) @ np.asarray(w_fc, np.float32)
           + np.asarray(b_fc, np.float32))

    shared = {
        "wq": wq, "wk": wk, "wv": wv, "bqk": bqk,
        "wap": np.asarray(w_attnproj, np.float32).astype(bf),
        "wfc": wfc.astype(bf),
        "bfc": bfc.astype(np.float32),
        "wpj": np.asarray(w_proj, np.float32).astype(bf),
    }
    in_maps = []
    for core in range(8):
        b, half = core // 2, core % 2
        xb = x[b]
        own = xb[half * TO:(half + 1) * TO]
        other = xb[(1 - half) * TO:(2 - half) * TO]
        m = dict(shared)
        m["x"] = np.ascontiguousarray(np.concatenate([own, other], 0))
        in_maps.append(m)
    return in_maps


def kernel(x, ln1_g, ln1_b, w_qkv, b_qkv, w_attnproj, b_attnproj,
           ln2_g, ln2_b, w_fc, b_fc, w_proj, b_proj):
    global LAST_RESULT
    in_maps = prepare_in_maps(
        x, ln1_g, ln1_b, w_qkv, b_qkv, w_attnproj, b_attnproj,
        ln2_g, ln2_b, w_fc, b_fc, w_proj, b_proj)

    if "nc" not in _CACHE:
        _CACHE["nc"] = _build()
    nc = _CACHE["nc"]

    LAST_RESULT = run_bass_kernel_spmd(nc, in_maps, core_ids=list(range(8)))

    out = np.empty((4, T, C), np.float32)
    for core in range(8):
        b, half = core // 2, core % 2
        out[b, half * TO:(half + 1) * TO] = LAST_RESULT.results[core]["out"]
    return out

